# revision 42
# baseline (speedup 1.0000x reference)
"""Causal self-attention Bass kernel for Trainium2, 8-core SPMD (v17).

Sharding: core k = 4*b + g  (b = batch 0/1, g = head-group of 4 heads).
Each core computes, for its batch b and heads 4g..4g+3:
    qkv      = xT.T @ w_attn[:, cols(g)]   (x pre-transposed fp16 on host)
    S^T      = K^T.T Q^T                   (fp16, head-pair row-tiled)
    att      = exp(S^T/8 - ln32)           (ACT engine; merged 1024-wide
                                            activations on full k-tiles)
    y_unnormT, sumexp = [V | 1].T @ att    (ones-row trick; causal boundary
                                            tiles get a triangle mask on gpsimd)
    yT       = y_unnormT * (1/sumexp)      (per-hp broadcast of recip sumexp)
    partial  = yT.T @ w_proj(rows(g), :)
Host sums the 4 partials per batch and adds b_proj + b_v @ w_proj.

Optimizations vs the 224us v3 baseline (now ~164us):
 - S matmuls of consecutive k-tile PAIRS emitted adjacently (and attV
   pairs likewise), halving PE array-mode switches (full-array <->
   64-row-tiled) and their drains; attV lags one pair so the exp and
   gpsimd-mask hops are fully off the critical path (at16 bufs=8).
 - input DMAs split per-ci on two issue queues (sync: wqkv, scalar: xs)
   so the first qkv matmul starts ~5us earlier; xs prefetched 2 chunks
   ahead so chunk boundaries never wait on HBM.
 - chunk 0 qkv split head (q/k for hp0) / tail (v, hp1) so attention(0)
   and the exp pipeline start ~8us earlier.
 - pS merged to one [128,1024] tile; full k-tiles get ONE 1024-wide exp
   activation (fewer ACT instructions and sem hops).
 - psY single-buffered (per-hp CAST), freeing PSUM for the merged pS.
 - causal mask muls emitted right after exp (gpsimd hop off the attV
   critical path).
 - proj(qi) deferred to attention(qi+1, hp1) / injected mid-drive so its
   first matmul never head-of-line blocks the PE on the norm chain.
 - norm(2,hp1) and the final chunk use a PE-broadcast of sumexp (ones
   stationary at partition 64) + sliced DVE reciprocal instead of the
   ~10us 4-hop DRAM bounce; tail pipelines recip slices with the output
   projection per t-tile, front-loads the hp0 proj partials, and casts
   PSUM on the otherwise-idle scalar engine (keeps HAM warm).
PSUM: pS 2x[128,1024] + mm 2x[128,512] + psY 1x[65,1024] = 16KB exactly.
"""

import numpy as np

import concourse.bass as bass
import concourse.mybir as mybir
import concourse.tile as tile

F32 = mybir.dt.float32
F16 = mybir.dt.float16
AFT = mybir.ActivationFunctionType

T = 2048      # sequence length
C = 1024      # model dim
D = 64        # head dim
HPC = 4       # heads per core
JQ = HPC * D  # per-core q (or k, or v) width = 256
TK = T // 128    # 16 t-tiles
CK = C // 128    # 8 c-tiles
NCH = T // 512   # 4 chunks
LN32 = 3.4657359027997265


def split_multiwaits(nc):
    """This container's walrus rejects >1 sem-wait per instruction.
    Split extras into single-wait EventSemaphore stubs on the same engine."""
    n = 0
    cnt = [0]
    for fn in nc.m.functions:
        for bb in fn.blocks:
            out = None
            for idx, ins in enumerate(bb.instructions):
                si = ins.sync_info
                if si is not None and si.on_wait and len(si.on_wait) > 1:
                    if out is None:
                        out = list(bb.instructions[:idx])
                    waits = list(si.on_wait)
                    n += 1
                    for w in waits[:-1]:
                        cnt[0] += 1
                        out.append(
                            mybir.InstEventSemaphore(
                                name=f"mwsplit-{cnt[0]}",
                                opcode="EventSemaphore",
                                engine=ins.engine,
                                ins=[],
                                outs=[],
                                sync_info=mybir.SyncInfo(on_wait=[w], on_update=[]),
                            )
                        )
                    ins.sync_info = mybir.SyncInfo(
                        on_wait=[waits[-1]], on_update=list(si.on_update or [])
                    )
                    out.append(ins)
                elif out is not None:
                    out.append(ins)
            if out is not None:
                bb.instructions = out
    return n


def build_nc():
    nc = bass.Bass()
    # host pre-laid-out: xs_d[n, p, ci*512+t2] = x[512n+t2, 128ci+p]
    xs_d = nc.dram_tensor("xs", [NCH, 128, CK * 512], F16, kind="ExternalInput")
    # wqkv_d[p, ci*768+m] = w_attn_slice[128ci+p, m]
    wqkv_d = nc.dram_tensor("wqkv", [128, CK * 3 * JQ], F16, kind="ExternalInput")
    # wp_d[p, a*1024+m] = w_proj_slice[128a+p, m]
    wp_d = nc.dram_tensor("wp", [128, 2 * C], F16, kind="ExternalInput")
    mask_d = nc.dram_tensor("mask", [128, 128], F16, kind="ExternalInput")
    out_d = nc.dram_tensor("out", [T, C], F16, kind="ExternalOutput")

    with tile.TileContext(nc) as tc:
        with (
            tc.tile_pool(name="const", bufs=1) as constp,
            tc.tile_pool(name="persist", bufs=1) as persist,
            tc.tile_pool(name="xs", bufs=2) as xsp,
            tc.tile_pool(name="att16", bufs=8) as att16p,
            tc.tile_pool(name="ystp", bufs=2) as ystp,
            tc.tile_pool(name="ytnp", bufs=2) as ytnp,
            tc.tile_pool(name="osbp", bufs=3) as osbp,
            tc.tile_pool(name="bcp", bufs=4) as bcp,
            tc.tile_pool(name="rdr", bufs=4, space="DRAM") as rdrp,
            tc.tile_pool(name="ps", bufs=2, space="PSUM") as psp,
            tc.tile_pool(name="psy", bufs=1, space="PSUM") as psyp,
        ):
            ebias = constp.tile([128, 1], F32)
            nc.vector.memset(ebias[:], -LN32)
            ones64 = constp.tile([128, 64], F16)
            nc.gpsimd.memset(ones64[:], 1.0)

            # interleave per-ci weight + x chunk-0 loads so qkv(0) can start
            # after ~2 small transfers instead of the full 3MB
            wqkv_r = persist.tile([128, CK, 3 * JQ], F16)
            wqkv_v = wqkv_d.rearrange("p (a m) -> p a m", a=CK)
            xs_c = {}
            xs_c[0] = xsp.tile([128, CK, 512], F16, tag="xs", name="xs_0")
            xs_v = [xs_d[n].rearrange("p (a t) -> p a t", a=CK) for n in range(NCH)]
            mask_sb = constp.tile([128, 128], F16)
            # two issue queues so descriptor issue (~600ns each) pipelines;
            # mask load deferred behind the first ci (not needed until the
            # first diagonal attV)
            # q/k weight cols + x on two queues (gate the first S);
            # v weight cols + wp on gpsimd (not needed until the first
            # diagonal attVs), so S(0) waits on 2MB instead of 2.5MB
            for ci in range(CK):
                nc.sync.dma_start(
                    out=wqkv_r[:, ci, 0 : 2 * JQ], in_=wqkv_v[:, ci, 0 : 2 * JQ]
                )
                nc.scalar.dma_start(out=xs_c[0][:, ci, :], in_=xs_v[0][:, ci, :])
                nc.gpsimd.dma_start(
                    out=wqkv_r[:, ci, 2 * JQ :], in_=wqkv_v[:, ci, 2 * JQ :]
                )
                if ci == 0:
                    nc.sync.dma_start(out=mask_sb[:], in_=mask_d[:])
            wp_r = persist.tile([128, 2, C], F16)
            nc.gpsimd.dma_start(
                out=wp_r[:], in_=wp_d.rearrange("p (a m) -> p a m", a=2)
            )

            def fetch_xs(n, eng=None):
                eng = eng or nc.sync
                xs_c[n] = xsp.tile([128, CK, 512], F16, tag="xs", name=f"xs_{n}")
                for ci in range(CK):
                    eng.dma_start(
                        out=xs_c[n][:, ci, :], in_=xs_v[n][:, ci, :]
                    )

            fetch_xs(1, eng=nc.scalar)

            # per-(jt, chunk) q/k tiles; jt: 0,1 = q (hp0, hp1), 2,3 = k
            qkT = {
                (jt, n): persist.tile(
                    [128, 512], F16, tag=f"qkT_{jt}_{n}", name=f"qkT_{jt}_{n}"
                )
                for jt in range(4)
                for n in range(NCH)
            }
            # v natural per t-tile: 4 heads x [64 v-cols + ones col]
            v16 = [
                persist.tile([128, HPC, 65], F16, tag=f"v16_{ti}", name=f"v16_{ti}")
                for ti in range(TK)
            ]
            for ti in range(TK):
                nc.vector.tensor_copy(
                    v16[ti][:, :, 64],
                    mask_sb[:, 127:128].broadcast_to([128, HPC]),
                )

            # ---------------- filler generators (<=2 matmuls per step) -----
            def gen_qkv(n, part="all", vy=2):
                def jt_steps(jt):
                    ps = psp.tile([128, 512], F32, tag="mm", name="psqk")
                    for ci in range(CK):
                        nc.tensor.matmul(
                            ps[:],
                            wqkv_r[:, ci, jt * 128 : (jt + 1) * 128],
                            xs_c[n][:, ci, :],
                            start=(ci == 0),
                            stop=(ci == CK - 1),
                        )
                        if ci % 2 == 1:
                            yield
                    nc.vector.tensor_copy(qkT[jt, n][:], ps[:])
                    yield

                def v_steps(tl):
                    ti = 4 * n + tl
                    psv = psp.tile([128, JQ], F32, tag="mm", name="psv")
                    for ci in range(CK):
                        nc.tensor.matmul(
                            psv[:],
                            xs_c[n][:, ci, tl * 128 : (tl + 1) * 128],
                            wqkv_r[:, ci, 2 * JQ : 3 * JQ],
                            start=(ci == 0),
                            stop=(ci == CK - 1),
                        )
                        if ci % vy == vy - 1:
                            yield
                    nc.vector.tensor_copy(
                        v16[ti][:, :, 0:64],
                        psv.rearrange("p (h e) -> p h e", h=HPC),
                    )
                    yield

                # earliest-needed first: q/k for hp0 + v (diag attVs need
                # v16 emitted before them), then hp1 tiles
                if part == "head":
                    subs = (
                        jt_steps(0), jt_steps(2),
                        v_steps(0), v_steps(1), v_steps(2), v_steps(3),
                    )
                elif part == "tail":
                    subs = (jt_steps(1), jt_steps(3))
                else:
                    subs = (
                        jt_steps(0), jt_steps(2),
                        v_steps(0), v_steps(1), v_steps(2), v_steps(3),
                        jt_steps(1), jt_steps(3),
                    )
                for sub in subs:
                    yield from sub

            def gen_proj(qi, yTn):
                for tl in range(4):
                    ti = 4 * qi + tl
                    osb = osbp.tile([128, 1024], F16, tag="osb", name="osb")
                    for n2 in range(2):
                        psO = psp.tile([128, 512], F32, tag="mm", name="psO")
                        for hp in range(2):
                            nc.tensor.matmul(
                                psO[:],
                                yTn[:, hp, tl * 128 : (tl + 1) * 128],
                                wp_r[:, hp, n2 * 512 : (n2 + 1) * 512],
                                start=(hp == 0),
                                stop=(hp == 1),
                            )
                        yield
                        nc.vector.tensor_copy(
                            osb[:, n2 * 512 : (n2 + 1) * 512], psO[:]
                        )
                        yield
                    nc.sync.dma_start(
                        out=out_d[ti * 128 : (ti + 1) * 128, :], in_=osb[:]
                    )

            # ---------------- attention (ACT-paced; lag-1 attV) ------------
            def gen_attention(qi, hp, ySt, psY):
                qt = qkT[hp, qi]
                nki = 4 * qi + 4
                pend = []  # [(ki, att tile)] awaiting attV (lag one pair)

                def emit_attv(ki, at16):
                    d = ki - 4 * qi
                    off = 128 * d if d >= 0 else 0
                    for e in range(2):
                        nc.tensor.matmul(
                            psY[:, 512 * e + off : 512 * e + 512],
                            v16[ki][:, 2 * hp + e, :],
                            at16[:, 512 * e + off : 512 * e + 512],
                            start=(ki == 0),
                            stop=(ki == nki - 1),
                        )

                def emit_s(ki):
                    # S pair for one k-tile (row-tiled 64x128 matmuls);
                    # pairs of consecutive ki are emitted adjacently so the
                    # PE array-mode switch (full <-> row-tiled) happens once
                    # per pair instead of once per ki
                    kt = qkT[2 + hp, ki // 4]
                    kl = (ki % 4) * 128
                    d = ki - 4 * qi
                    off = 128 * d if d >= 0 else 0
                    at16 = att16p.tile([128, 1024], F16, tag="att16", name="at16")
                    pS = psp.tile([128, 1024], F32, tag="pS", name="pS")
                    for e in range(2):
                        nc.tensor.matmul(
                            pS[:, 512 * e + off : 512 * e + 512],
                            kt[64 * e : 64 * e + 64, kl : kl + 128],
                            qt[64 * e : 64 * e + 64, off:512],
                            start=True,
                            stop=True,
                        )
                    return (off, pS, at16)

                def emit_exp(ki, off, pS, at16):
                    d = ki - 4 * qi
                    if off == 0:
                        nc.scalar.activation(
                            at16[:, :],
                            pS[:, :],
                            AFT.Exp,
                            scale=0.125,
                            bias=ebias[:],
                        )
                    else:
                        for e in range(2):
                            nc.scalar.activation(
                                at16[:, 512 * e + off : 512 * e + 512],
                                pS[:, 512 * e + off : 512 * e + 512],
                                AFT.Exp,
                                scale=0.125,
                                bias=ebias[:],
                            )
                    if d >= 0:
                        avw = at16.rearrange("p (g f) -> p g f", g=2)
                        nc.gpsimd.tensor_mul(
                            avw[:, :, off : off + 128],
                            avw[:, :, off : off + 128],
                            mask_sb[:].unsqueeze(1).broadcast_to([128, 2, 128]),
                        )

                for kp in range(0, nki, 2):
                    s0 = emit_s(kp)
                    s1 = emit_s(kp + 1)
                    yield
                    emit_exp(kp, *s0)
                    yield
                    emit_exp(kp + 1, *s1)
                    yield
                    for p in pend:
                        emit_attv(*p)
                    pend = [(kp, s0[2]), (kp + 1, s1[2])]
                    yield
                for p in pend:
                    emit_attv(*p)
                nc.vector.tensor_copy(
                    ySt[:, 2 * hp : 2 * hp + 2, :],
                    psY.rearrange("p (e f) -> p e f", e=2),
                )

            def gen_norm_fast(qi, hp, ySt, yTn):
                """PE-broadcast sumexp (no DMA hops) + per-tl reciprocal
                slices and norm muls on DVE. Run as a filler so the two PE
                matmuls land after the ySt cast completes (no head-of-line
                block at the chunk boundary)."""
                for _ in range(6):
                    yield
                psB = psp.tile([128, 512], F32, tag="mm", name="psB")
                for e in range(2):
                    nc.tensor.matmul(
                        psB[64 * e : 64 * e + 64, :],
                        ones64[64:65, :],
                        ySt[64:65, 2 * hp + e, :],
                        start=True,
                        stop=True,
                    )
                yield
                rbc = bcp.tile([128, 512], F32, tag="rbc", name="rbc")
                rbc2 = bcp.tile([64, 512], F32, tag="rbc2", name="rbc2")
                for tl in range(4):
                    sl = slice(tl * 128, tl * 128 + 128)
                    nc.vector.reciprocal(rbc[:, sl], psB[:, sl])
                    nc.vector.tensor_copy(rbc2[:, sl], rbc[64:128, sl])
                    nc.vector.tensor_mul(
                        yTn[0:64, hp, sl], ySt[0:64, 2 * hp, sl], rbc[0:64, sl]
                    )
                    nc.vector.tensor_mul(
                        yTn[64:128, hp, sl], ySt[0:64, 2 * hp + 1, sl], rbc2[:, sl]
                    )

            def emit_tail(qi, ySt, yTn):
                """Last chunk, hp1: keep the PE streaming through the norm
                chain by front-loading proj's hp0-partial matmuls (hp0's
                yTn has been ready since mid-chunk), then per-tl pipeline
                of reciprocal -> muls -> hp1-accumulate -> cast (on the
                idle scalar engine) -> out DMA."""
                # hp0 partials for tl 0/1 in pS-tag psum (free at tail)
                psO2 = []
                for g in range(2):
                    ps2 = psp.tile([128, 1024], F32, tag="pS", name=f"psO2_{g}")
                    psO2.append(ps2)
                    for n2 in range(2):
                        nc.tensor.matmul(
                            ps2[:, n2 * 512 : (n2 + 1) * 512],
                            yTn[:, 0, g * 128 : (g + 1) * 128],
                            wp_r[:, 0, n2 * 512 : (n2 + 1) * 512],
                            start=True,
                            stop=False,
                        )
                psB = psp.tile([128, 512], F32, tag="mm", name="psB")
                for e in range(2):
                    nc.tensor.matmul(
                        psB[64 * e : 64 * e + 64, :],
                        ones64[64:65, :],
                        ySt[64:65, 2 + e, :],
                        start=True,
                        stop=True,
                    )
                rbc = bcp.tile([128, 512], F32, tag="rbc", name="rbc")
                rbc2 = bcp.tile([64, 512], F32, tag="rbc2", name="rbc2")
                for tl in range(4):
                    sl = slice(tl * 128, tl * 128 + 128)
                    nc.vector.reciprocal(rbc[:, sl], psB[:, sl])
                    nc.vector.tensor_copy(rbc2[:, sl], rbc[64:128, sl])
                    nc.vector.tensor_mul(
                        yTn[0:64, 1, sl], ySt[0:64, 2, sl], rbc[0:64, sl]
                    )
                    nc.vector.tensor_mul(
                        yTn[64:128, 1, sl], ySt[0:64, 3, sl], rbc2[:, sl]
                    )
                    ti = 4 * qi + tl
                    osb = osbp.tile([128, 1024], F16, tag="osb", name="osb")
                    for n2 in range(2):
                        if tl < 2:
                            psO = psO2[tl][:, n2 * 512 : (n2 + 1) * 512]
                            nc.tensor.matmul(
                                psO,
                                yTn[:, 1, tl * 128 : (tl + 1) * 128],
                                wp_r[:, 1, n2 * 512 : (n2 + 1) * 512],
                                start=False,
                                stop=True,
                            )
                        else:
                            psO = psp.tile([128, 512], F32, tag="mm", name="psO")
                            for hp2 in range(2):
                                nc.tensor.matmul(
                                    psO[:],
                                    yTn[:, hp2, tl * 128 : (tl + 1) * 128],
                                    wp_r[:, hp2, n2 * 512 : (n2 + 1) * 512],
                                    start=(hp2 == 0),
                                    stop=(hp2 == 1),
                                )
                            psO = psO[:]
                        nc.scalar.activation(
                            osb[:, n2 * 512 : (n2 + 1) * 512], psO, AFT.Copy
                        )
                    nc.sync.dma_start(
                        out=out_d[ti * 128 : (ti + 1) * 128, :], in_=osb[:]
                    )

            def emit_norm_hp(qi, hp, ySt, yTn):
                # per-hp reciprocal-broadcast via DRAM bounce (4 DMAs),
                # hidden under the next ~half chunk of attention
                s_dr = rdrp.tile([1, 2 * 512], F16, tag="s_dr", name="s_dr")
                nc.sync.dma_start(
                    out=s_dr[:],
                    in_=ySt[64:65, 2 * hp : 2 * hp + 2, :].rearrange(
                        "p a f -> p (a f)"
                    ),
                )
                sp = bcp.tile([128, 8], F16, tag="sp", name="sp")
                nc.sync.dma_start(
                    out=sp[:], in_=s_dr.rearrange("a (p j) -> p (a j)", p=128)
                )
                rp = bcp.tile([128, 8], F32, tag="rp", name="rp")
                nc.vector.reciprocal(rp[:], sp[:])
                rp16 = bcp.tile([128, 8], F16, tag="rp16", name="rp16")
                nc.vector.tensor_copy(rp16[:], rp[:])
                r_dr = rdrp.tile([128, 8], F16, tag="r_dr", name="r_dr")
                nc.sync.dma_start(out=r_dr[:], in_=rp16[:])
                bc = bcp.tile([64, 2, 512], F16, tag="bc", name="bc")
                nc.gpsimd.dma_start(
                    out=bc[:],
                    in_=r_dr.rearrange("p j -> (p j)")[None, :]
                    .to_broadcast([64, 2 * 512])
                    .rearrange("p (a f) -> p a f", a=2),
                )
                for e in range(2):
                    j = 2 * hp + e
                    nc.vector.tensor_mul(
                        yTn[64 * e : 64 * e + 64, hp, :],
                        ySt[0:64, j, :],
                        bc[:, e, :],
                    )

            def drive(gen, fillers, inject=None):
                """Run gen; pull one step from each filler at each yield.
                inject: {pull_index: generator} appended mid-drive."""
                pulls = 0
                for _ in gen:
                    pulls += 1
                    if inject and pulls in inject:
                        fillers.append(inject[pulls])
                    for f in list(fillers):
                        if next(f, StopIteration) is StopIteration:
                            fillers.remove(f)

            def drain(gen, fillers):
                for _ in gen:
                    for f in list(fillers):
                        if next(f, StopIteration) is StopIteration:
                            fillers.remove(f)

            # ---------------- main schedule --------------------------------
            # chunk 0: q/k for hp0 first, then start attention(0) with the
            # rest of qkv(0) interleaved (v tiles land just ahead of the
            # diagonal attVs; vy=4 keeps them ahead of emission order)
            fillers = []
            for _ in gen_qkv(0, part="head"):
                pass
            tail0 = gen_qkv(0, part="tail", vy=4)
            fillers.append(tail0)

            qkv_gen = None
            pend_proj = None  # proj(qi-1), deferred so its first matmul
            # never head-of-line blocks the PE queue on the norm DMA chain
            for qi in range(NCH):
                # correctness: chunk qi's qkT/v16 producers must be fully
                # emitted before attention(qi) instructions reference them
                if qkv_gen is not None:
                    drain(qkv_gen, fillers)
                    qkv_gen = None
                if qi + 2 < NCH:
                    fetch_xs(qi + 2)
                ySt = ystp.tile([65, 4, 512], F16, tag="ySt", name=f"ySt_{qi}")
                yTn = ytnp.tile([128, 2, 512], F16, tag="yTn", name=f"yTn_{qi}")
                last = qi == NCH - 1
                for hp in range(2):
                    psY = psyp.tile(
                        [65, 1024], F32, tag="psY", name=f"psY_{qi}_{hp}"
                    )
                    if hp == 1 and qi == 0:
                        drain(tail0, fillers)  # jt1/jt3 of chunk 0
                    if hp == 0 and qi + 1 < NCH:
                        qkv_gen = gen_qkv(qi + 1)
                        fillers.append(qkv_gen)
                    inject = None
                    if hp == 1 and pend_proj is not None:
                        fillers.append(pend_proj)
                        pend_proj = None
                    elif hp == 0 and last and pend_proj is not None:
                        # attention(3,hp0) has no qkv filler; feed it proj(2)
                        # once the norm(2,hp1) fast chain has landed
                        inject = {30: pend_proj}
                        pend_proj = None
                    drive(gen_attention(qi, hp, ySt, psY), fillers, inject)
                    if qkv_gen is not None and qkv_gen not in fillers:
                        qkv_gen = None  # already exhausted
                    if last and hp == 1:
                        emit_tail(qi, ySt, yTn)
                    elif qi == 2 and hp == 1:
                        fillers.append(gen_norm_fast(qi, hp, ySt, yTn))
                    else:
                        emit_norm_hp(qi, hp, ySt, yTn)
                if not last:
                    pend_proj = gen_proj(qi, yTn)
            for f in fillers:
                for _ in f:
                    pass

    split_multiwaits(nc)
    return nc


def make_mask():
    p = np.arange(128)[:, None]
    f = np.arange(128)[None, :]
    return (p <= f).astype(np.float16)


def shard_inputs(x, w_attn, b_attn, w_proj):
    """Returns per-core input maps (8 cores: core = 4*b + g).

    Tensors are pre-laid-out so every device DMA is contiguous per
    partition:
      xs[n, p, ci*512+t2]   = x[b][512n+t2, 128ci+p]
      wqkv[p, ci*768+m]     = [wq|wk|wv][128ci+p, m]
      wp[p, a*1024+m]       = w_proj_slice[128a+p, m]
    """
    mask = make_mask()
    in_maps = []
    # x[b].T -> [ci, p, n, t2] -> [n, p, ci, t2]
    xs16 = [
        np.ascontiguousarray(
            x[b].T.reshape(CK, 128, NCH, 512).transpose(2, 1, 0, 3)
            .reshape(NCH, 128, CK * 512)
        ).astype(np.float16)
        for b in range(2)
    ]
    for core in range(8):
        b, g = divmod(core, 4)
        wq = w_attn[:, g * JQ : (g + 1) * JQ]
        wk = w_attn[:, C + g * JQ : C + (g + 1) * JQ]
        wv = w_attn[:, 2 * C + g * JQ : 2 * C + (g + 1) * JQ]
        wqkv = np.concatenate([wq, wk, wv], axis=1)  # [C, 768]
        wqkv_r = np.ascontiguousarray(
            wqkv.reshape(CK, 128, 3 * JQ).transpose(1, 0, 2).reshape(128, -1)
        )
        wp = w_proj[g * JQ : (g + 1) * JQ, :]  # [256, C]
        wp_r = np.ascontiguousarray(
            wp.reshape(2, 128, C).transpose(1, 0, 2).reshape(128, -1)
        )
        in_maps.append(
            {
                "xs": xs16[b],
                "wqkv": wqkv_r.astype(np.float16),
                "wp": wp_r.astype(np.float16),
                "mask": mask,
            }
        )
    return in_maps


def combine_outputs(results, b_attn, w_proj, b_proj):
    """Sum per-head-group partials per batch; add bias corrections."""
    corr = b_attn[2 * C :] @ w_proj + b_proj  # v-bias pushthrough + proj bias
    out = np.zeros((2, T, C), dtype=np.float32)
    for core in range(8):
        b = core // 4
        out[b] += results[core]["out"].astype(np.float32)
    out += corr[None, None, :].astype(np.float32)
    return out


# ---------------------------------------------------------------------------
# harness entry point
# ---------------------------------------------------------------------------
_NC_CACHE = []


def _get_nc():
    if not _NC_CACHE:
        _NC_CACHE.append(build_nc())
    return _NC_CACHE[0]


def _run(in_maps, trace=False, tmpdir=None):
    from concourse import bass_utils

    return bass_utils.run_bass_kernel_spmd(
        _get_nc(), in_maps, core_ids=list(range(8)), trace=trace, tmpdir=tmpdir
    )


def kernel(x, w_attn, b_attn, w_proj, b_proj):
    """Full-input causal self-attention on 8 NeuronCores.

    x: [2, 2048, 1024] f32; w_attn: [1024, 3072]; b_attn: [3072];
    w_proj: [1024, 1024]; b_proj: [1024].  Returns [2, 2048, 1024] f32.
    """
    x = np.asarray(x, dtype=np.float32)
    w_attn = np.asarray(w_attn, dtype=np.float32)
    b_attn = np.asarray(b_attn, dtype=np.float32)
    w_proj = np.asarray(w_proj, dtype=np.float32)
    b_proj = np.asarray(b_proj, dtype=np.float32)

    in_maps = shard_inputs(x, w_attn, b_attn, w_proj)
    res = _run(in_maps)
    return combine_outputs(res.results, b_attn, w_proj, b_proj)


# revision 43
# speedup vs baseline: 1.0024x; 1.0024x over previous
"""Causal self-attention Bass kernel for Trainium2, 8-core SPMD (v17).

Sharding: core k = 4*b + g  (b = batch 0/1, g = head-group of 4 heads).
Each core computes, for its batch b and heads 4g..4g+3:
    qkv      = xT.T @ w_attn[:, cols(g)]   (x pre-transposed fp16 on host)
    S^T      = K^T.T Q^T                   (fp16, head-pair row-tiled)
    att      = exp(S^T/8 - ln32)           (ACT engine; merged 1024-wide
                                            activations on full k-tiles)
    y_unnormT, sumexp = [V | 1].T @ att    (ones-row trick; causal boundary
                                            tiles get a triangle mask on gpsimd)
    yT       = y_unnormT * (1/sumexp)      (per-hp broadcast of recip sumexp)
    partial  = yT.T @ w_proj(rows(g), :)
Host sums the 4 partials per batch and adds b_proj + b_v @ w_proj.

Optimizations vs the 224us v3 baseline (now ~164us):
 - S matmuls of consecutive k-tile PAIRS emitted adjacently (and attV
   pairs likewise), halving PE array-mode switches (full-array <->
   64-row-tiled) and their drains; attV lags one pair so the exp and
   gpsimd-mask hops are fully off the critical path (at16 bufs=8).
 - input DMAs split per-ci on three issue queues (sync: q/k weight
   cols, scalar: xs, gpsimd: v weight cols + wp, which are not needed
   until the first diagonal attVs) so the first S matmul is gated by
   2MB instead of 2.5MB; xs prefetched 2 chunks ahead.
 - chunk 0 qkv split head (q/k for hp0) / tail (v, hp1) so attention(0)
   and the exp pipeline start ~8us earlier.
 - pS merged to one [128,1024] tile; full k-tiles get ONE 1024-wide exp
   activation (fewer ACT instructions and sem hops).
 - psY single-buffered (per-hp CAST), freeing PSUM for the merged pS.
 - causal mask muls emitted right after exp (gpsimd hop off the attV
   critical path).
 - proj(qi) deferred to attention(qi+1, hp1) / injected mid-drive so its
   first matmul never head-of-line blocks the PE on the norm chain.
 - norm(2,hp1) and the final chunk use a PE-broadcast of sumexp (ones
   stationary at partition 64) + sliced DVE reciprocal instead of the
   ~10us 4-hop DRAM bounce; tail pipelines recip slices with the output
   projection per t-tile, front-loads the hp0 proj partials, and casts
   PSUM on the otherwise-idle scalar engine (keeps HAM warm).
PSUM: pS 2x[128,1024] + mm 2x[128,512] + psY 1x[65,1024] = 16KB exactly.
"""

import numpy as np

import concourse.bass as bass
import concourse.mybir as mybir
import concourse.tile as tile

F32 = mybir.dt.float32
F16 = mybir.dt.float16
AFT = mybir.ActivationFunctionType

T = 2048      # sequence length
C = 1024      # model dim
D = 64        # head dim
HPC = 4       # heads per core
JQ = HPC * D  # per-core q (or k, or v) width = 256
TK = T // 128    # 16 t-tiles
CK = C // 128    # 8 c-tiles
NCH = T // 512   # 4 chunks
LN32 = 3.4657359027997265


def split_multiwaits(nc):
    """This container's walrus rejects >1 sem-wait per instruction.
    Split extras into single-wait EventSemaphore stubs on the same engine."""
    n = 0
    cnt = [0]
    for fn in nc.m.functions:
        for bb in fn.blocks:
            out = None
            for idx, ins in enumerate(bb.instructions):
                si = ins.sync_info
                if si is not None and si.on_wait and len(si.on_wait) > 1:
                    if out is None:
                        out = list(bb.instructions[:idx])
                    waits = list(si.on_wait)
                    n += 1
                    for w in waits[:-1]:
                        cnt[0] += 1
                        out.append(
                            mybir.InstEventSemaphore(
                                name=f"mwsplit-{cnt[0]}",
                                opcode="EventSemaphore",
                                engine=ins.engine,
                                ins=[],
                                outs=[],
                                sync_info=mybir.SyncInfo(on_wait=[w], on_update=[]),
                            )
                        )
                    ins.sync_info = mybir.SyncInfo(
                        on_wait=[waits[-1]], on_update=list(si.on_update or [])
                    )
                    out.append(ins)
                elif out is not None:
                    out.append(ins)
            if out is not None:
                bb.instructions = out
    return n


def build_nc():
    nc = bass.Bass()
    # host pre-laid-out: xs_d[n, p, ci*512+t2] = x[512n+t2, 128ci+p]
    xs_d = nc.dram_tensor("xs", [NCH, 128, CK * 512], F16, kind="ExternalInput")
    # wqkv_d[p, ci*768+m] = w_attn_slice[128ci+p, m]
    wqkv_d = nc.dram_tensor("wqkv", [128, CK * 3 * JQ], F16, kind="ExternalInput")
    # wp_d[p, a*1024+m] = w_proj_slice[128a+p, m]
    wp_d = nc.dram_tensor("wp", [128, 2 * C], F16, kind="ExternalInput")
    mask_d = nc.dram_tensor("mask", [128, 128], F16, kind="ExternalInput")
    out_d = nc.dram_tensor("out", [T, C], F16, kind="ExternalOutput")

    with tile.TileContext(nc) as tc:
        with (
            tc.tile_pool(name="const", bufs=1) as constp,
            tc.tile_pool(name="persist", bufs=1) as persist,
            tc.tile_pool(name="xs", bufs=2) as xsp,
            tc.tile_pool(name="att16", bufs=8) as att16p,
            tc.tile_pool(name="ystp", bufs=2) as ystp,
            tc.tile_pool(name="ytnp", bufs=2) as ytnp,
            tc.tile_pool(name="osbp", bufs=3) as osbp,
            tc.tile_pool(name="bcp", bufs=4) as bcp,
            tc.tile_pool(name="rdr", bufs=4, space="DRAM") as rdrp,
            tc.tile_pool(name="ps", bufs=2, space="PSUM") as psp,
            tc.tile_pool(name="psy", bufs=1, space="PSUM") as psyp,
        ):
            ebias = constp.tile([128, 1], F32)
            nc.vector.memset(ebias[:], -LN32)
            ones64 = constp.tile([128, 64], F16)
            nc.gpsimd.memset(ones64[:], 1.0)

            # interleave per-ci weight + x chunk-0 loads so qkv(0) can start
            # after ~2 small transfers instead of the full 3MB
            wqkv_r = persist.tile([128, CK, 3 * JQ], F16)
            wqkv_v = wqkv_d.rearrange("p (a m) -> p a m", a=CK)
            xs_c = {}
            xs_c[0] = xsp.tile([128, CK, 512], F16, tag="xs", name="xs_0")
            xs_v = [xs_d[n].rearrange("p (a t) -> p a t", a=CK) for n in range(NCH)]
            mask_sb = constp.tile([128, 128], F16)
            # two issue queues so descriptor issue (~600ns each) pipelines;
            # mask load deferred behind the first ci (not needed until the
            # first diagonal attV)
            # q/k weight cols + x on two queues (gate the first S);
            # v weight cols + wp on gpsimd (not needed until the first
            # diagonal attVs), so S(0) waits on 2MB instead of 2.5MB
            for ci in range(CK):
                nc.sync.dma_start(
                    out=wqkv_r[:, ci, 0 : 2 * JQ], in_=wqkv_v[:, ci, 0 : 2 * JQ]
                )
                nc.scalar.dma_start(out=xs_c[0][:, ci, :], in_=xs_v[0][:, ci, :])
                nc.gpsimd.dma_start(
                    out=wqkv_r[:, ci, 2 * JQ :], in_=wqkv_v[:, ci, 2 * JQ :]
                )
                if ci == 0:
                    nc.sync.dma_start(out=mask_sb[:], in_=mask_d[:])
            wp_r = persist.tile([128, 2, C], F16)
            nc.gpsimd.dma_start(
                out=wp_r[:], in_=wp_d.rearrange("p (a m) -> p a m", a=2)
            )

            def fetch_xs(n, eng=None):
                eng = eng or nc.sync
                xs_c[n] = xsp.tile([128, CK, 512], F16, tag="xs", name=f"xs_{n}")
                for ci in range(CK):
                    eng.dma_start(
                        out=xs_c[n][:, ci, :], in_=xs_v[n][:, ci, :]
                    )

            fetch_xs(1, eng=nc.scalar)

            # per-(jt, chunk) q/k tiles; jt: 0,1 = q (hp0, hp1), 2,3 = k
            qkT = {
                (jt, n): persist.tile(
                    [128, 512], F16, tag=f"qkT_{jt}_{n}", name=f"qkT_{jt}_{n}"
                )
                for jt in range(4)
                for n in range(NCH)
            }
            # v natural per t-tile: 4 heads x [64 v-cols + ones col]
            v16 = [
                persist.tile([128, HPC, 65], F16, tag=f"v16_{ti}", name=f"v16_{ti}")
                for ti in range(TK)
            ]
            for ti in range(TK):
                nc.vector.tensor_copy(
                    v16[ti][:, :, 64],
                    mask_sb[:, 127:128].broadcast_to([128, HPC]),
                )

            # ---------------- filler generators (<=2 matmuls per step) -----
            def gen_qkv(n, part="all", vy=2):
                def jt_steps(jt):
                    ps = psp.tile([128, 512], F32, tag="mm", name="psqk")
                    for ci in range(CK):
                        nc.tensor.matmul(
                            ps[:],
                            wqkv_r[:, ci, jt * 128 : (jt + 1) * 128],
                            xs_c[n][:, ci, :],
                            start=(ci == 0),
                            stop=(ci == CK - 1),
                        )
                        if ci % 2 == 1:
                            yield
                    nc.vector.tensor_copy(qkT[jt, n][:], ps[:])
                    yield

                def v_steps(tl):
                    ti = 4 * n + tl
                    psv = psp.tile([128, JQ], F32, tag="mm", name="psv")
                    for ci in range(CK):
                        nc.tensor.matmul(
                            psv[:],
                            xs_c[n][:, ci, tl * 128 : (tl + 1) * 128],
                            wqkv_r[:, ci, 2 * JQ : 3 * JQ],
                            start=(ci == 0),
                            stop=(ci == CK - 1),
                        )
                        if ci % vy == vy - 1:
                            yield
                    nc.vector.tensor_copy(
                        v16[ti][:, :, 0:64],
                        psv.rearrange("p (h e) -> p h e", h=HPC),
                    )
                    yield

                # earliest-needed first: q/k for hp0 + v (diag attVs need
                # v16 emitted before them), then hp1 tiles
                if part == "head":
                    subs = (
                        jt_steps(0), jt_steps(2),
                        v_steps(0), v_steps(1), v_steps(2), v_steps(3),
                    )
                elif part == "tail":
                    subs = (jt_steps(1), jt_steps(3))
                else:
                    subs = (
                        jt_steps(0), jt_steps(2),
                        v_steps(0), v_steps(1), v_steps(2), v_steps(3),
                        jt_steps(1), jt_steps(3),
                    )
                for sub in subs:
                    yield from sub

            def gen_proj(qi, yTn):
                for tl in range(4):
                    ti = 4 * qi + tl
                    osb = osbp.tile([128, 1024], F16, tag="osb", name="osb")
                    for n2 in range(2):
                        psO = psp.tile([128, 512], F32, tag="mm", name="psO")
                        for hp in range(2):
                            nc.tensor.matmul(
                                psO[:],
                                yTn[:, hp, tl * 128 : (tl + 1) * 128],
                                wp_r[:, hp, n2 * 512 : (n2 + 1) * 512],
                                start=(hp == 0),
                                stop=(hp == 1),
                            )
                        yield
                        nc.vector.tensor_copy(
                            osb[:, n2 * 512 : (n2 + 1) * 512], psO[:]
                        )
                        yield
                    nc.sync.dma_start(
                        out=out_d[ti * 128 : (ti + 1) * 128, :], in_=osb[:]
                    )

            # ---------------- attention (ACT-paced; lag-1 attV) ------------
            def gen_attention(qi, hp, ySt, psY):
                qt = qkT[hp, qi]
                nki = 4 * qi + 4
                pend = []  # [(ki, att tile)] awaiting attV (lag one pair)

                def emit_attv(ki, at16):
                    d = ki - 4 * qi
                    off = 128 * d if d >= 0 else 0
                    for e in range(2):
                        nc.tensor.matmul(
                            psY[:, 512 * e + off : 512 * e + 512],
                            v16[ki][:, 2 * hp + e, :],
                            at16[:, 512 * e + off : 512 * e + 512],
                            start=(ki == 0),
                            stop=(ki == nki - 1),
                        )

                def emit_s(ki):
                    # S pair for one k-tile (row-tiled 64x128 matmuls);
                    # pairs of consecutive ki are emitted adjacently so the
                    # PE array-mode switch (full <-> row-tiled) happens once
                    # per pair instead of once per ki
                    kt = qkT[2 + hp, ki // 4]
                    kl = (ki % 4) * 128
                    d = ki - 4 * qi
                    off = 128 * d if d >= 0 else 0
                    at16 = att16p.tile([128, 1024], F16, tag="att16", name="at16")
                    pS = psp.tile([128, 1024], F32, tag="pS", name="pS")
                    for e in range(2):
                        nc.tensor.matmul(
                            pS[:, 512 * e + off : 512 * e + 512],
                            kt[64 * e : 64 * e + 64, kl : kl + 128],
                            qt[64 * e : 64 * e + 64, off:512],
                            start=True,
                            stop=True,
                        )
                    return (off, pS, at16)

                def emit_exp(ki, off, pS, at16):
                    d = ki - 4 * qi
                    if off == 0:
                        nc.scalar.activation(
                            at16[:, :],
                            pS[:, :],
                            AFT.Exp,
                            scale=0.125,
                            bias=ebias[:],
                        )
                    else:
                        for e in range(2):
                            nc.scalar.activation(
                                at16[:, 512 * e + off : 512 * e + 512],
                                pS[:, 512 * e + off : 512 * e + 512],
                                AFT.Exp,
                                scale=0.125,
                                bias=ebias[:],
                            )
                    if d >= 0:
                        avw = at16.rearrange("p (g f) -> p g f", g=2)
                        nc.gpsimd.tensor_mul(
                            avw[:, :, off : off + 128],
                            avw[:, :, off : off + 128],
                            mask_sb[:].unsqueeze(1).broadcast_to([128, 2, 128]),
                        )

                for kp in range(0, nki, 2):
                    s0 = emit_s(kp)
                    s1 = emit_s(kp + 1)
                    yield
                    emit_exp(kp, *s0)
                    yield
                    emit_exp(kp + 1, *s1)
                    yield
                    for p in pend:
                        emit_attv(*p)
                    pend = [(kp, s0[2]), (kp + 1, s1[2])]
                    yield
                for p in pend:
                    emit_attv(*p)
                nc.vector.tensor_copy(
                    ySt[:, 2 * hp : 2 * hp + 2, :],
                    psY.rearrange("p (e f) -> p e f", e=2),
                )

            def emit_norm_fast(qi, hp, ySt, yTn):
                """PE-broadcast sumexp (no DMA hops) + per-tl reciprocal
                slices and norm muls on DVE. Chain latency ~3us vs ~10us
                for the DRAM bounce; costs 2 PE matmuls."""
                psB = psp.tile([128, 512], F32, tag="mm", name="psB")
                for e in range(2):
                    nc.tensor.matmul(
                        psB[64 * e : 64 * e + 64, :],
                        ones64[64:65, :],
                        ySt[64:65, 2 * hp + e, :],
                        start=True,
                        stop=True,
                    )
                rbc = bcp.tile([128, 512], F32, tag="rbc", name="rbc")
                rbc2 = bcp.tile([64, 512], F32, tag="rbc2", name="rbc2")
                for tl in range(4):
                    sl = slice(tl * 128, tl * 128 + 128)
                    nc.vector.reciprocal(rbc[:, sl], psB[:, sl])
                    nc.vector.tensor_copy(rbc2[:, sl], rbc[64:128, sl])
                    nc.vector.tensor_mul(
                        yTn[0:64, hp, sl], ySt[0:64, 2 * hp, sl], rbc[0:64, sl]
                    )
                    nc.vector.tensor_mul(
                        yTn[64:128, hp, sl], ySt[0:64, 2 * hp + 1, sl], rbc2[:, sl]
                    )

            def emit_tail(qi, ySt, yTn):
                """Last chunk, hp1: keep the PE streaming through the norm
                chain by front-loading proj's hp0-partial matmuls (hp0's
                yTn has been ready since mid-chunk), then per-tl pipeline
                of reciprocal -> muls -> hp1-accumulate -> cast (on the
                idle scalar engine) -> out DMA."""
                # hp0 partials for tl 0/1 in pS-tag psum (free at tail)
                psO2 = []
                for g in range(2):
                    ps2 = psp.tile([128, 1024], F32, tag="pS", name=f"psO2_{g}")
                    psO2.append(ps2)
                    for n2 in range(2):
                        nc.tensor.matmul(
                            ps2[:, n2 * 512 : (n2 + 1) * 512],
                            yTn[:, 0, g * 128 : (g + 1) * 128],
                            wp_r[:, 0, n2 * 512 : (n2 + 1) * 512],
                            start=True,
                            stop=False,
                        )
                psB = psp.tile([128, 512], F32, tag="mm", name="psB")
                for e in range(2):
                    nc.tensor.matmul(
                        psB[64 * e : 64 * e + 64, :],
                        ones64[64:65, :],
                        ySt[64:65, 2 + e, :],
                        start=True,
                        stop=True,
                    )
                rbc = bcp.tile([128, 512], F32, tag="rbc", name="rbc")
                rbc2 = bcp.tile([64, 512], F32, tag="rbc2", name="rbc2")
                for tl in range(4):
                    sl = slice(tl * 128, tl * 128 + 128)
                    nc.vector.reciprocal(rbc[:, sl], psB[:, sl])
                    nc.vector.tensor_copy(rbc2[:, sl], rbc[64:128, sl])
                    nc.vector.tensor_mul(
                        yTn[0:64, 1, sl], ySt[0:64, 2, sl], rbc[0:64, sl]
                    )
                    nc.vector.tensor_mul(
                        yTn[64:128, 1, sl], ySt[0:64, 3, sl], rbc2[:, sl]
                    )
                    ti = 4 * qi + tl
                    osb = osbp.tile([128, 1024], F16, tag="osb", name="osb")
                    for n2 in range(2):
                        if tl < 2:
                            psO = psO2[tl][:, n2 * 512 : (n2 + 1) * 512]
                            nc.tensor.matmul(
                                psO,
                                yTn[:, 1, tl * 128 : (tl + 1) * 128],
                                wp_r[:, 1, n2 * 512 : (n2 + 1) * 512],
                                start=False,
                                stop=True,
                            )
                        else:
                            psO = psp.tile([128, 512], F32, tag="mm", name="psO")
                            for hp2 in range(2):
                                nc.tensor.matmul(
                                    psO[:],
                                    yTn[:, hp2, tl * 128 : (tl + 1) * 128],
                                    wp_r[:, hp2, n2 * 512 : (n2 + 1) * 512],
                                    start=(hp2 == 0),
                                    stop=(hp2 == 1),
                                )
                            psO = psO[:]
                        nc.scalar.activation(
                            osb[:, n2 * 512 : (n2 + 1) * 512], psO, AFT.Copy
                        )
                    nc.sync.dma_start(
                        out=out_d[ti * 128 : (ti + 1) * 128, :], in_=osb[:]
                    )

            def emit_norm_hp(qi, hp, ySt, yTn):
                # per-hp reciprocal-broadcast via DRAM bounce (4 DMAs),
                # hidden under the next ~half chunk of attention
                s_dr = rdrp.tile([1, 2 * 512], F16, tag="s_dr", name="s_dr")
                nc.sync.dma_start(
                    out=s_dr[:],
                    in_=ySt[64:65, 2 * hp : 2 * hp + 2, :].rearrange(
                        "p a f -> p (a f)"
                    ),
                )
                sp = bcp.tile([128, 8], F16, tag="sp", name="sp")
                nc.sync.dma_start(
                    out=sp[:], in_=s_dr.rearrange("a (p j) -> p (a j)", p=128)
                )
                rp = bcp.tile([128, 8], F32, tag="rp", name="rp")
                nc.vector.reciprocal(rp[:], sp[:])
                rp16 = bcp.tile([128, 8], F16, tag="rp16", name="rp16")
                nc.vector.tensor_copy(rp16[:], rp[:])
                r_dr = rdrp.tile([128, 8], F16, tag="r_dr", name="r_dr")
                nc.sync.dma_start(out=r_dr[:], in_=rp16[:])
                bc = bcp.tile([64, 2, 512], F16, tag="bc", name="bc")
                nc.gpsimd.dma_start(
                    out=bc[:],
                    in_=r_dr.rearrange("p j -> (p j)")[None, :]
                    .to_broadcast([64, 2 * 512])
                    .rearrange("p (a f) -> p a f", a=2),
                )
                for e in range(2):
                    j = 2 * hp + e
                    nc.vector.tensor_mul(
                        yTn[64 * e : 64 * e + 64, hp, :],
                        ySt[0:64, j, :],
                        bc[:, e, :],
                    )

            def drive(gen, fillers, inject=None):
                """Run gen; pull one step from each filler at each yield.
                inject: {pull_index: generator} appended mid-drive."""
                pulls = 0
                for _ in gen:
                    pulls += 1
                    if inject and pulls in inject:
                        fillers.append(inject[pulls])
                    for f in list(fillers):
                        if next(f, StopIteration) is StopIteration:
                            fillers.remove(f)

            def drain(gen, fillers):
                for _ in gen:
                    for f in list(fillers):
                        if next(f, StopIteration) is StopIteration:
                            fillers.remove(f)

            # ---------------- main schedule --------------------------------
            # chunk 0: q/k for hp0 first, then start attention(0) with the
            # rest of qkv(0) interleaved (v tiles land just ahead of the
            # diagonal attVs; vy=4 keeps them ahead of emission order)
            fillers = []
            for _ in gen_qkv(0, part="head"):
                pass
            tail0 = gen_qkv(0, part="tail", vy=4)
            fillers.append(tail0)

            qkv_gen = None
            pend_proj = None  # proj(qi-1), deferred so its first matmul
            # never head-of-line blocks the PE queue on the norm DMA chain
            for qi in range(NCH):
                # correctness: chunk qi's qkT/v16 producers must be fully
                # emitted before attention(qi) instructions reference them
                if qkv_gen is not None:
                    drain(qkv_gen, fillers)
                    qkv_gen = None
                if qi + 2 < NCH:
                    fetch_xs(qi + 2)
                ySt = ystp.tile([65, 4, 512], F16, tag="ySt", name=f"ySt_{qi}")
                yTn = ytnp.tile([128, 2, 512], F16, tag="yTn", name=f"yTn_{qi}")
                last = qi == NCH - 1
                for hp in range(2):
                    psY = psyp.tile(
                        [65, 1024], F32, tag="psY", name=f"psY_{qi}_{hp}"
                    )
                    if hp == 1 and qi == 0:
                        drain(tail0, fillers)  # jt1/jt3 of chunk 0
                    if hp == 0 and qi + 1 < NCH:
                        qkv_gen = gen_qkv(qi + 1)
                        fillers.append(qkv_gen)
                    inject = None
                    if hp == 1 and pend_proj is not None:
                        fillers.append(pend_proj)
                        pend_proj = None
                    elif hp == 0 and last and pend_proj is not None:
                        # attention(3,hp0) has no qkv filler; feed it proj(2)
                        # once the norm(2,hp1) fast chain has landed
                        inject = {30: pend_proj}
                        pend_proj = None
                    drive(gen_attention(qi, hp, ySt, psY), fillers, inject)
                    if qkv_gen is not None and qkv_gen not in fillers:
                        qkv_gen = None  # already exhausted
                    if last and hp == 1:
                        emit_tail(qi, ySt, yTn)
                    elif qi == 2 and hp == 1:
                        emit_norm_fast(qi, hp, ySt, yTn)
                    else:
                        emit_norm_hp(qi, hp, ySt, yTn)
                if not last:
                    pend_proj = gen_proj(qi, yTn)
            for f in fillers:
                for _ in f:
                    pass

    split_multiwaits(nc)
    return nc


def make_mask():
    p = np.arange(128)[:, None]
    f = np.arange(128)[None, :]
    return (p <= f).astype(np.float16)


def shard_inputs(x, w_attn, b_attn, w_proj):
    """Returns per-core input maps (8 cores: core = 4*b + g).

    Tensors are pre-laid-out so every device DMA is contiguous per
    partition:
      xs[n, p, ci*512+t2]   = x[b][512n+t2, 128ci+p]
      wqkv[p, ci*768+m]     = [wq|wk|wv][128ci+p, m]
      wp[p, a*1024+m]       = w_proj_slice[128a+p, m]
    """
    mask = make_mask()
    in_maps = []
    # x[b].T -> [ci, p, n, t2] -> [n, p, ci, t2]
    xs16 = [
        np.ascontiguousarray(
            x[b].T.reshape(CK, 128, NCH, 512).transpose(2, 1, 0, 3)
            .reshape(NCH, 128, CK * 512)
        ).astype(np.float16)
        for b in range(2)
    ]
    for core in range(8):
        b, g = divmod(core, 4)
        wq = w_attn[:, g * JQ : (g + 1) * JQ]
        wk = w_attn[:, C + g * JQ : C + (g + 1) * JQ]
        wv = w_attn[:, 2 * C + g * JQ : 2 * C + (g + 1) * JQ]
        wqkv = np.concatenate([wq, wk, wv], axis=1)  # [C, 768]
        wqkv_r = np.ascontiguousarray(
            wqkv.reshape(CK, 128, 3 * JQ).transpose(1, 0, 2).reshape(128, -1)
        )
        wp = w_proj[g * JQ : (g + 1) * JQ, :]  # [256, C]
        wp_r = np.ascontiguousarray(
            wp.reshape(2, 128, C).transpose(1, 0, 2).reshape(128, -1)
        )
        in_maps.append(
            {
                "xs": xs16[b],
                "wqkv": wqkv_r.astype(np.float16),
                "wp": wp_r.astype(np.float16),
                "mask": mask,
            }
        )
    return in_maps


def combine_outputs(results, b_attn, w_proj, b_proj):
    """Sum per-head-group partials per batch; add bias corrections."""
    corr = b_attn[2 * C :] @ w_proj + b_proj  # v-bias pushthrough + proj bias
    out = np.zeros((2, T, C), dtype=np.float32)
    for core in range(8):
        b = core // 4
        out[b] += results[core]["out"].astype(np.float32)
    out += corr[None, None, :].astype(np.float32)
    return out


# ---------------------------------------------------------------------------
# harness entry point
# ---------------------------------------------------------------------------
_NC_CACHE = []


def _get_nc():
    if not _NC_CACHE:
        _NC_CACHE.append(build_nc())
    return _NC_CACHE[0]


def _run(in_maps, trace=False, tmpdir=None):
    from concourse import bass_utils

    return bass_utils.run_bass_kernel_spmd(
        _get_nc(), in_maps, core_ids=list(range(8)), trace=trace, tmpdir=tmpdir
    )


def kernel(x, w_attn, b_attn, w_proj, b_proj):
    """Full-input causal self-attention on 8 NeuronCores.

    x: [2, 2048, 1024] f32; w_attn: [1024, 3072]; b_attn: [3072];
    w_proj: [1024, 1024]; b_proj: [1024].  Returns [2, 2048, 1024] f32.
    """
    x = np.asarray(x, dtype=np.float32)
    w_attn = np.asarray(w_attn, dtype=np.float32)
    b_attn = np.asarray(b_attn, dtype=np.float32)
    w_proj = np.asarray(w_proj, dtype=np.float32)
    b_proj = np.asarray(b_proj, dtype=np.float32)

    in_maps = shard_inputs(x, w_attn, b_attn, w_proj)
    res = _run(in_maps)
    return combine_outputs(res.results, b_attn, w_proj, b_proj)
